# revision 27
# baseline (speedup 1.0000x reference)
"""Trainium2 Bass kernel for nn_DisentangledHierarchicalEncoder.

Strategy (8 NeuronCores, SPMD, zero collectives):
  The gather indices (seq_modify) are host-known, so the host pre-gathers the
  per-token raw features for each core's 6400 tokens (128 batch rows x 50) and
  pre-transposes everything to feature-major [feat, token] layout. Each core
  then runs a fully dense pipeline:
      content MLP (1024->1024->256->64) with input l2norm,
      text MLP (768->768->256->64) with input l2norm,
      cf linear (64->64), id passthrough,
      per-(token, modality) l2norm + LayerNorm (folded into one affine),
      4x4 self-attention (scores via G = 0.125 * wq.T @ wk), mean-pool,
  in 13 chunks of 512 tokens.  All matmuls run in float32r (TF32-like, 1
  cycle/row).  Per-token scalars are broadcast across partitions with PE
  outer-products against host-supplied pick matrices; partition-stacked pairs
  (content|text and cf|id) keep every engine op on 128 partitions.
"""

import numpy as np

NUM_ITEM = 50000
B, S, D = 1024, 50, 64
DC, DT = 1024, 768
N_CORES = 8
TOK = (B // N_CORES) * S          # 6400 real tokens per core
C = 512                           # chunk width (tokens per chunk)
NCH = 13                          # chunks per core
T = C * NCH                       # 6656 padded tokens per core
KC, KT_ = DC // 128, DT // 128    # k-tiles: 8 content, 6 text
LN_EPS = 1e-5

_CACHE = {}


def _build_consts():
    f32 = np.float32
    ones128 = np.ones((128, 1), f32)
    ones1 = np.ones((1, 128), f32)
    # stats lhsT [128, 8]. First matmul uses cols 0:4 on emb_ct (rows 0,1 of
    # st), second uses cols 4:8 on emb_cfid; within that slice cols 2,3 must
    # be the hot ones so the sums land on st rows 2,3.
    hotab = np.zeros((128, 8), f32)
    hotab[0:64, 0] = 1.0      # st row 0 <- sum over partitions 0:64 (c)
    hotab[64:128, 1] = 1.0    # st row 1 (t)
    hotab[0:64, 4 + 2] = 1.0  # st row 2 (cf)
    hotab[64:128, 4 + 3] = 1.0  # st row 3 (id)
    # score lhsT: 4 variants [128, 8]; variant u has slice-local cols
    # (2u, 2u+1) hot for (top, bottom) halves.
    hotp = np.zeros((128, 32), f32)
    for u in range(4):
        hotp[0:64, 8 * u + 2 * u] = 1.0
        hotp[64:128, 8 * u + 2 * u + 1] = 1.0
    # pick: [4, 256]; cols 0:128 broadcast rows 0/1 to halves, cols 128:256 rows 2/3
    pickp = np.zeros((4, 256), f32)
    pickp[0, 0:64] = 1.0
    pickp[1, 64:128] = 1.0
    pickp[2, 128 + 0:128 + 64] = 1.0
    pickp[3, 128 + 64:128 + 128] = 1.0
    # sumexp lhsT [8, 8]: first mm cols 0:4 on e_ct (se rows 0,1), second mm
    # cols 4:8 on e_cfid (slice-local cols 2,3 hot -> se rows 2,3).
    quads = np.zeros((8, 8), f32)
    quads[0:4, 0] = 1.0
    quads[4:8, 1] = 1.0
    quads[0:4, 4 + 2] = 1.0
    quads[4:8, 4 + 3] = 1.0
    # r replicate lhsT: [4, 16]; cols j (j<8): one-hot row j//4 (for e_ct pair);
    # cols 8+j: one-hot row 2 + j//4 (for e_cfid pair)
    reps = np.zeros((4, 16), f32)
    for j in range(8):
        reps[j // 4, j] = 1.0
        reps[2 + j // 4, 8 + j] = 1.0
    # a4 lhsT: [8, 4]; col n hot at rows {n, 4+n}
    nsums = np.zeros((8, 4), f32)
    for n in range(4):
        nsums[n, n] = 1.0
        nsums[4 + n, n] = 1.0
    return dict(ones128=ones128, ones1=ones1, hotab=hotab, hotp=hotp,
                pickp=pickp, quads=quads, reps=reps, nsums=nsums)


def _build_nc(nch=NCH, n_cores=N_CORES):
    import concourse.bacc as bacc
    import concourse.tile as tile
    from concourse import mybir
    from contextlib import ExitStack

    T = C * nch
    FR = mybir.dt.float32r
    F32 = mybir.dt.float32
    AF = mybir.ActivationFunctionType

    nc = bacc.Bacc("TRN2", target_bir_lowering=False, debug=False,
                   num_devices=n_cores)

    din = {}
    def dt_in(name, shape, dt=FR):
        din[name] = nc.dram_tensor(name, list(shape), dt, kind="ExternalInput")
        return din[name]

    xc = dt_in("xc", [DC, T])
    xt = dt_in("xt", [DT, T])
    xcf = dt_in("xcf", [64, T])
    xid = dt_in("xid", [64, T])
    cw1 = dt_in("cw1", [DC, DC])
    cw2 = dt_in("cw2", [DC, 256])
    tw1 = dt_in("tw1", [DT, DT])
    tw2 = dt_in("tw2", [DT, 256])
    w3p = dt_in("w3p", [128, 4, 128])
    cfwp = dt_in("cfwp", [64, 128])
    g2t = dt_in("g2t", [128, 128])
    g2b = dt_in("g2b", [128, 128])
    wv2 = dt_in("wv2", [128, 64])
    b1c = dt_in("b1c", [KC, 128], F32)
    b2c = dt_in("b2c", [2, 128], F32)
    b1t = dt_in("b1t", [KT_, 128], F32)
    b2t = dt_in("b2t", [2, 128], F32)
    b3 = dt_in("b3", [128, 1], F32)
    bcf = dt_in("bcf", [64, 1], F32)
    ones128 = dt_in("ones128", [128, 1])
    ones1 = dt_in("ones1", [1, 128])
    hotab = dt_in("hotab", [128, 8])
    hotp = dt_in("hotp", [128, 32])
    pickp = dt_in("pickp", [4, 256])
    quads = dt_in("quads", [8, 8])
    reps = dt_in("reps", [4, 16])
    nsums = dt_in("nsums", [8, 4])
    out = nc.dram_tensor("out", [64, T], F32, kind="ExternalOutput")

    xc_r = xc.rearrange("(kt p) t -> p kt t", p=128)
    xt_r = xt.rearrange("(kt p) t -> p kt t", p=128)

    with nc.allow_low_precision("float32r tiles feed float32r matmuls by design"), \
            tile.TileContext(nc) as tc:
        with ExitStack() as ctx:
            wp = ctx.enter_context(tc.tile_pool(name="wp", bufs=1))
            xin = ctx.enter_context(tc.tile_pool(name="xin", bufs=1))
            h1p = ctx.enter_context(tc.tile_pool(name="h1p", bufs=1))
            h2p = ctx.enter_context(tc.tile_pool(name="h2p", bufs=1))
            sqp = ctx.enter_context(tc.tile_pool(name="sqp", bufs=2))
            tmpp = ctx.enter_context(tc.tile_pool(name="tmpp", bufs=2))
            embp = ctx.enter_context(tc.tile_pool(name="embp", bufs=2))
            xnp = ctx.enter_context(tc.tile_pool(name="xnp", bufs=2))
            tinyp = ctx.enter_context(tc.tile_pool(name="tinyp", bufs=4))
            outp = ctx.enter_context(tc.tile_pool(name="outp", bufs=2))
            pbig = ctx.enter_context(tc.tile_pool(name="pbig", bufs=4,
                                                  space="PSUM"))
            psmall = ctx.enter_context(tc.tile_pool(name="psmall", bufs=4,
                                                    space="PSUM"))

            # ---- resident weights / consts ----
            cw1s = wp.tile([128, KC, DC], FR)
            nc.sync.dma_start(out=cw1s, in_=cw1.rearrange("(kt p) m -> p kt m", p=128))
            cw2s = wp.tile([128, KC, 256], FR)
            nc.sync.dma_start(out=cw2s, in_=cw2.rearrange("(kt p) m -> p kt m", p=128))
            tw1s = wp.tile([128, KT_, DT], FR)
            nc.sync.dma_start(out=tw1s, in_=tw1.rearrange("(kt p) m -> p kt m", p=128))
            tw2s = wp.tile([128, KT_, 256], FR)
            nc.sync.dma_start(out=tw2s, in_=tw2.rearrange("(kt p) m -> p kt m", p=128))
            w3ps = wp.tile([128, 4, 128], FR)
            nc.sync.dma_start(out=w3ps, in_=w3p[:, :, :])
            cfwps = wp.tile([64, 128], FR)
            nc.sync.dma_start(out=cfwps, in_=cfwp[:, :])
            g2ts = wp.tile([128, 128], FR)
            nc.sync.dma_start(out=g2ts, in_=g2t[:, :])
            g2bs = wp.tile([128, 128], FR)
            nc.sync.dma_start(out=g2bs, in_=g2b[:, :])
            wv2s = wp.tile([128, 64], FR)
            nc.sync.dma_start(out=wv2s, in_=wv2[:, :])
            b1cs = wp.tile([128, KC], F32)
            nc.sync.dma_start(out=b1cs, in_=b1c.rearrange("m p -> p m"))
            b2cs = wp.tile([128, 2], F32)
            nc.sync.dma_start(out=b2cs, in_=b2c.rearrange("m p -> p m"))
            b1ts = wp.tile([128, KT_], F32)
            nc.sync.dma_start(out=b1ts, in_=b1t.rearrange("m p -> p m"))
            b2ts = wp.tile([128, 2], F32)
            nc.sync.dma_start(out=b2ts, in_=b2t.rearrange("m p -> p m"))
            b3s = wp.tile([128, 1], F32)
            nc.sync.dma_start(out=b3s, in_=b3[:, :])
            bcfs = wp.tile([64, 1], F32)
            nc.sync.dma_start(out=bcfs, in_=bcf[:, :])
            o128 = wp.tile([128, 1], FR)
            nc.sync.dma_start(out=o128, in_=ones128[:, :])
            o1 = wp.tile([1, 128], FR)
            nc.sync.dma_start(out=o1, in_=ones1[:, :])
            hotabs = wp.tile([128, 8], FR)
            nc.sync.dma_start(out=hotabs, in_=hotab[:, :])
            hotps = wp.tile([128, 32], FR)
            nc.sync.dma_start(out=hotps, in_=hotp[:, :])
            pickps = wp.tile([4, 256], FR)
            nc.sync.dma_start(out=pickps, in_=pickp[:, :])
            quadss = wp.tile([8, 8], FR)
            nc.sync.dma_start(out=quadss, in_=quads[:, :])
            repss = wp.tile([4, 16], FR)
            nc.sync.dma_start(out=repss, in_=reps[:, :])
            nsumss = wp.tile([8, 4], FR)
            nc.sync.dma_start(out=nsumss, in_=nsums[:, :])

            from concourse.alu_op_type import AluOpType as ALU

            state = {}

            def make_units(j):
                """PE-dense MLP work for chunk j, as a list of emit fns."""
                sl = slice(j * C, (j + 1) * C)
                ctx_j = {}

                def u_load():
                    xc_j = xin.tile([128, KC, C], FR, tag="xc")
                    nc.sync.dma_start(out=xc_j, in_=xc_r[:, :, sl])
                    xt_j = xin.tile([128, KT_, C], FR, tag="xt")
                    nc.sync.dma_start(out=xt_j, in_=xt_r[:, :, sl])
                    xcf_j = xin.tile([64, C], FR, tag="xcf")
                    nc.sync.dma_start(out=xcf_j, in_=xcf[:, sl])
                    emb_cfid = embp.tile([128, C], FR, tag="emb_cfid")
                    nc.sync.dma_start(out=emb_cfid[64:128, :], in_=xid[:, sl])
                    ctx_j.update(xc_j=xc_j, xt_j=xt_j, xcf_j=xcf_j,
                                 emb_cfid=emb_cfid)

                def norm_ss(xj, kt, tag):
                    # squares on GPSIMD, ss via ones-matmul, inv = 1/max(sqrt)
                    ss_ps = psmall.tile([1, C], F32, tag="sm")
                    for k in range(kt):
                        sq_k = sqp.tile([128, C], FR, tag="sq", name="sq_k")
                        nc.gpsimd.tensor_mul(sq_k, xj[:, k, :], xj[:, k, :])
                        nc.tensor.matmul(ss_ps[:, :], o128[:, :], sq_k,
                                         start=(k == 0), stop=(k == kt - 1))
                    nrm = tinyp.tile([1, C], FR, tag="t1c", bufs=3)
                    nc.scalar.activation(out=nrm, in_=ss_ps[:, :], func=AF.Sqrt)
                    ncl = tinyp.tile([1, C], FR, tag="t1c", bufs=3)
                    nc.vector.tensor_scalar_max(ncl, nrm, 1e-12)
                    inv = tinyp.tile([1, C], FR, tag="t1c", bufs=3)
                    nc.vector.reciprocal(inv, ncl)
                    ctx_j["inv" + tag] = inv

                def norm_ib(tag):
                    # broadcast inv to [128, C] (emitted after L1 m0 so the
                    # sqrt/max/recip latency hides under matmul work)
                    ib_ps = pbig.tile([128, C], F32, tag="mm")
                    nc.tensor.matmul(ib_ps[:, :], o1[:, :],
                                     ctx_j["inv" + tag][:, :],
                                     start=True, stop=True)
                    invb = tmpp.tile([128, C], FR, tag="invb")
                    nc.vector.tensor_copy(invb, ib_ps[:, :])
                    ctx_j["invb" + tag] = invb

                def l1_tile(xj_key, kt, w1s, b1sT, tag, m):
                    def evac(ps, mi):
                        h1 = ctx_j["h1" + tag]
                        tmp = tmpp.tile([128, C], FR, tag="tmp", name="tmp")
                        nc.vector.tensor_mul(tmp, ps[:, :],
                                             ctx_j["invb" + tag])
                        nc.scalar.activation(out=h1[:, mi, :], in_=tmp,
                                             func=AF.Relu,
                                             bias=b1sT[:, mi:mi + 1])

                    def emit():
                        xj = ctx_j[xj_key]
                        if m == 0:
                            h1 = h1p.tile([128, kt, C], FR, tag="h1" + tag)
                            ctx_j["h1" + tag] = h1
                        ps = pbig.tile([128, C], F32, tag="mm", name="ps")
                        for k in range(kt):
                            nc.tensor.matmul(
                                ps[:, :], w1s[:, k, 128 * m:128 * (m + 1)],
                                xj[:, k, :], start=(k == 0), stop=(k == kt - 1))
                        if m == 0:
                            ctx_j["ps0" + tag] = ps
                            return
                        if m == 1:
                            norm_ib(tag)
                            evac(ctx_j.pop("ps0" + tag), 0)
                        evac(ps, m)
                    return emit

                def l2_tile(kt, w2s, b2sT, tag, m):
                    def emit():
                        h1 = ctx_j["h1" + tag]
                        if m == 0:
                            h2 = h2p.tile([128, 2, C], FR, tag="h2" + tag)
                            ctx_j["h2" + tag] = h2
                        h2 = ctx_j["h2" + tag]
                        ps = pbig.tile([128, C], F32, tag="mm")
                        for k in range(kt):
                            nc.tensor.matmul(
                                ps[:, :], w2s[:, k, 128 * m:128 * (m + 1)],
                                h1[:, k, :], start=(k == 0), stop=(k == kt - 1))
                        nc.scalar.activation(out=h2[:, m, :], in_=ps[:, :],
                                             func=AF.Relu,
                                             bias=b2sT[:, m:m + 1])
                    return emit

                def u_l3cf():
                    h2c, h2t = ctx_j["h2c"], ctx_j["h2t"]
                    ps3 = pbig.tile([128, C], F32, tag="mm")
                    nc.tensor.matmul(ps3[:, :], w3ps[:, 0, :], h2c[:, 0, :],
                                     start=True, stop=False)
                    nc.tensor.matmul(ps3[:, :], w3ps[:, 1, :], h2c[:, 1, :],
                                     start=False, stop=False)
                    nc.tensor.matmul(ps3[:, :], w3ps[:, 2, :], h2t[:, 0, :],
                                     start=False, stop=False)
                    nc.tensor.matmul(ps3[:, :], w3ps[:, 3, :], h2t[:, 1, :],
                                     start=False, stop=True)
                    emb_ct = embp.tile([128, C], FR, tag="emb_ct")
                    nc.scalar.activation(out=emb_ct, in_=ps3[:, :],
                                         func=AF.Identity, bias=b3s[:, :])
                    pcf = pbig.tile([128, C], F32, tag="mm")
                    nc.tensor.matmul(pcf[:, :], cfwps[:, :], ctx_j["xcf_j"],
                                     start=True, stop=True)
                    emb_cfid = ctx_j["emb_cfid"]
                    nc.scalar.activation(out=emb_cfid[0:64, :],
                                         in_=pcf[0:64, :],
                                         func=AF.Identity, bias=bcfs[:, :])
                    state[j] = (emb_ct, emb_cfid)

                units = [u_load,
                         lambda: norm_ss(ctx_j["xc_j"], KC, "c")]
                units += [l1_tile("xc_j", KC, cw1s, b1cs, "c", m)
                          for m in range(KC)]
                units += [lambda: norm_ss(ctx_j["xt_j"], KT_, "t")]
                units += [l2_tile(KC, cw2s, b2cs, "c", m) for m in range(2)]
                units += [l1_tile("xt_j", KT_, tw1s, b1ts, "t", m)
                          for m in range(KT_)]
                units += [l2_tile(KT_, tw2s, b2ts, "t", m) for m in range(2)]
                units += [u_l3cf]
                return units

            def make_stages(j):
                """attention for chunk j (embs from state[j]), as emit fns."""
                sl = slice(j * C, (j + 1) * C)
                actx = {}

                def s_stats():
                    emb_ct, emb_cfid = state.pop(j)
                    actx["emb_ct"], actx["emb_cfid"] = emb_ct, emb_cfid
                    sq_ct = sqp.tile([128, C], FR, tag="sqs")
                    nc.gpsimd.tensor_mul(sq_ct, emb_ct, emb_ct)
                    sq_cfid = sqp.tile([128, C], FR, tag="sqs")
                    nc.gpsimd.tensor_mul(sq_cfid, emb_cfid, emb_cfid)
                    st_sum = psmall.tile([4, C], F32, tag="sm")
                    nc.tensor.matmul(st_sum[:, :], hotabs[:, 0:4], emb_ct,
                                     start=True, stop=False)
                    nc.tensor.matmul(st_sum[:, :], hotabs[:, 4:8], emb_cfid,
                                     start=False, stop=True)
                    st_ss = psmall.tile([4, C], F32, tag="sm")
                    nc.tensor.matmul(st_ss[:, :], hotabs[:, 0:4], sq_ct,
                                     start=True, stop=False)
                    nc.tensor.matmul(st_ss[:, :], hotabs[:, 4:8], sq_cfid,
                                     start=False, stop=True)
                    actx["st_sum"], actx["st_ss"] = st_sum, st_ss

                def s_ab():
                    # folded l2norm+LN: A = rsqrt(ss*(1/64+eps) - mu^2), B = mu*A
                    mu4 = tinyp.tile([4, C], FR, tag="t4c")
                    nc.vector.tensor_scalar_mul(mu4, actx["st_sum"][:, :],
                                                1.0 / 64)
                    musq = tinyp.tile([4, C], FR, tag="t4c")
                    nc.gpsimd.tensor_mul(musq, mu4, mu4)
                    apre = tinyp.tile([4, C], FR, tag="t4c")
                    nc.vector.scalar_tensor_tensor(
                        apre, actx["st_ss"][:, :], 1.0 / 64 + LN_EPS, musq,
                        op0=ALU.mult, op1=ALU.subtract)
                    asq = tinyp.tile([4, C], FR, tag="t4c")
                    nc.scalar.activation(out=asq, in_=apre, func=AF.Sqrt)
                    A4 = tinyp.tile([4, C], FR, tag="t4c")
                    nc.vector.reciprocal(A4, asq)
                    B4 = tinyp.tile([4, C], FR, tag="t4c")
                    nc.vector.tensor_mul(B4, mu4, A4)
                    actx["A4"], actx["B4"] = A4, B4

                def make_xn(ekey, pks, tag):
                    def emit():
                        abp = pbig.tile([128, C], F32, tag="mm")
                        nc.tensor.matmul(abp[:, :], pickps[:, pks],
                                         actx["A4"], start=True, stop=True)
                        bbp = pbig.tile([128, C], F32, tag="mm")
                        nc.tensor.matmul(bbp[:, :], pickps[:, pks],
                                         actx["B4"], start=True, stop=True)
                        tmp2 = tmpp.tile([128, C], FR, tag="tmp")
                        nc.vector.tensor_mul(tmp2, actx[ekey], abp[:, :])
                        xn = xnp.tile([128, C], FR, tag="xn" + tag)
                        nc.vector.tensor_sub(xn, tmp2, bbp[:, :])
                        actx["xn" + tag] = xn
                    return emit

                def score_a(i):
                    def emit():
                        if i == 0:
                            actx["s_ct"] = psmall.tile([8, C], F32, tag="sm",
                                                       name="s_ct")
                            actx["s_cfid"] = psmall.tile([8, C], F32, tag="sm",
                                                         name="s_cfid")
                        gsel = g2ts if i % 2 == 0 else g2bs
                        xn_src = actx["xnct"] if i < 2 else actx["xncfid"]
                        qg = pbig.tile([128, C], F32, tag="mm", name="qg")
                        nc.tensor.matmul(qg[:, :], gsel[:, :], xn_src,
                                         start=True, stop=True)
                        pr1 = tmpp.tile([128, C], FR, tag="pr", name="pr1")
                        nc.vector.tensor_mul(pr1, qg[:, :], actx["xnct"])
                        pr2 = tmpp.tile([128, C], FR, tag="pr", name="pr2")
                        nc.vector.tensor_mul(pr2, qg[:, :], actx["xncfid"])
                        actx["pr"] = (pr1, pr2)
                    return emit

                def score_b(i):
                    def emit():
                        pr1, pr2 = actx.pop("pr")
                        s_tile = actx["s_ct"] if i < 2 else actx["s_cfid"]
                        u = 2 * (i % 2)
                        nc.tensor.matmul(s_tile[:, :],
                                         hotps[:, 8 * u:8 * u + 8],
                                         pr1, start=(i % 2 == 0), stop=False)
                        nc.tensor.matmul(s_tile[:, :],
                                         hotps[:, 8 * (u + 1):8 * (u + 1) + 8],
                                         pr2, start=False, stop=(i % 2 == 1))
                    return emit

                def s_soft1():
                    e_ct = tmpp.tile([8, C], FR, tag="e_ct", bufs=1)
                    nc.scalar.activation(out=e_ct, in_=actx["s_ct"][:, :],
                                         func=AF.Exp)
                    e_cfid = tmpp.tile([8, C], FR, tag="e_cfid", bufs=1)
                    nc.scalar.activation(out=e_cfid, in_=actx["s_cfid"][:, :],
                                         func=AF.Exp)
                    se = psmall.tile([4, C], F32, tag="sm")
                    nc.tensor.matmul(se[:, :], quadss[:, 0:4], e_ct,
                                     start=True, stop=False)
                    nc.tensor.matmul(se[:, :], quadss[:, 4:8], e_cfid,
                                     start=False, stop=True)
                    tse = tinyp.tile([4, C], FR, tag="t4c")
                    nc.vector.tensor_scalar_mul(tse, se[:, :], 4.0)
                    rr = tinyp.tile([4, C], FR, tag="t4c")
                    nc.vector.reciprocal(rr, tse)
                    actx.update(e_ct=e_ct, e_cfid=e_cfid, rr=rr)

                def s_soft2():
                    w_pair = []
                    for pi, e_p in enumerate([actx["e_ct"], actx["e_cfid"]]):
                        rrep = pbig.tile([128, C], F32, tag="mm", name="rrep")
                        nc.tensor.matmul(rrep[0:8, :],
                                         repss[:, 8 * pi:8 * pi + 8],
                                         actx["rr"], start=True, stop=True)
                        w_p = tmpp.tile([8, C], FR, tag="w_p", name="w_p",
                                        bufs=2)
                        nc.vector.tensor_mul(w_p, e_p, rrep[0:8, :])
                        w_pair.append(w_p)
                    actx["w_pair"] = w_pair

                def s_out1():
                    w_pair = actx["w_pair"]
                    a4p = psmall.tile([4, C], F32, tag="sm")
                    nc.tensor.matmul(a4p[:, :], nsumss[:, :], w_pair[0],
                                     start=True, stop=False)
                    nc.tensor.matmul(a4p[:, :], nsumss[:, :], w_pair[1],
                                     start=False, stop=True)
                    a4s = tinyp.tile([4, C], FR, tag="t4c")
                    nc.vector.tensor_copy(a4s, a4p[:, :])
                    actx["a4s"] = a4s

                def s_out2():
                    a4s = actx["a4s"]
                    zps = []
                    for pi, xn_p in enumerate([actx["xnct"], actx["xncfid"]]):
                        arp = pbig.tile([128, C], F32, tag="mm", name="arp")
                        nc.tensor.matmul(
                            arp[:, :],
                            pickps[:, slice(128 * pi, 128 * (pi + 1))],
                            a4s, start=True, stop=True)
                        zp = tmpp.tile([128, C], FR, tag="pr", name="zp")
                        nc.vector.tensor_mul(zp, xn_p, arp[:, :])
                        zps.append(zp)
                    actx["zps"] = zps

                def s_out3():
                    fps = pbig.tile([128, C], F32, tag="mm")
                    for pi, zp in enumerate(actx["zps"]):
                        nc.tensor.matmul(fps[0:64, :], wv2s[:, :], zp,
                                         start=(pi == 0), stop=(pi == 1))
                    out_sb = outp.tile([64, C], F32, tag="osb")
                    nc.vector.tensor_copy(out_sb, fps[0:64, :])
                    nc.sync.dma_start(out=out[:, sl], in_=out_sb)

                return [s_stats, s_ab,
                        make_xn("emb_ct", slice(0, 128), "ct"),
                        make_xn("emb_cfid", slice(128, 256), "cfid"),
                        score_a(0), score_b(0), score_a(1), score_b(1),
                        score_a(2), score_b(2), score_a(3), score_b(3),
                        s_soft1, s_soft2, s_out1, s_out2, s_out3]

            for it in range(nch + 1):
                units = make_units(it) if it < nch else []
                stages = make_stages(it - 1) if it >= 1 else []
                # weave: distribute stages evenly among units
                n_u, n_s = len(units), len(stages)
                si = 0
                for ui, u in enumerate(units):
                    u()
                    want = ((ui + 1) * n_s) // max(n_u, 1)
                    while si < want:
                        stages[si]()
                        si += 1
                while si < n_s:
                    stages[si]()
                    si += 1

    nc.finalize()
    return nc


def _get_nc():
    if "nc" not in _CACHE:
        _CACHE["nc"] = _build_nc()
    return _CACHE["nc"]


def _host_prep(inputs, T=T):
    f32 = np.float32
    seq = np.asarray(inputs["seq_modify"])
    seq = np.where(seq == NUM_ITEM, 0, seq).astype(np.int64)  # [1024, 50]
    cf_full = np.asarray(inputs["content_feature"], dtype=f32)
    tf_full = np.asarray(inputs["text_feature"], dtype=f32)
    cff_full = np.asarray(inputs["cf_feature"], dtype=f32)
    ide_full = np.asarray(inputs["item_embeddings"], dtype=f32)

    c_w3 = np.asarray(inputs["c_w3"], dtype=f32)   # [64, 256]
    t_w3 = np.asarray(inputs["t_w3"], dtype=f32)
    cw3T = np.ascontiguousarray(c_w3.T)            # [256, 64]
    tw3T = np.ascontiguousarray(t_w3.T)
    w3p = np.zeros((128, 4, 128), f32)
    w3p[:, 0, 0:64] = cw3T[0:128]
    w3p[:, 1, 0:64] = cw3T[128:256]
    w3p[:, 2, 64:128] = tw3T[0:128]
    w3p[:, 3, 64:128] = tw3T[128:256]
    cf_w = np.asarray(inputs["cf_w"], dtype=f32)
    cfwp = np.zeros((64, 128), f32)
    cfwp[:, 0:64] = cf_w.T
    wq = np.asarray(inputs["wq"], dtype=f32)
    wk = np.asarray(inputs["wk"], dtype=f32)
    wv = np.asarray(inputs["wv"], dtype=f32)
    G = (wq.T @ wk) * (D ** -0.5)
    G2 = np.concatenate([G, G], axis=1)            # [64, 128]
    g2t = np.concatenate([G2, np.zeros((64, 128), f32)], axis=0)
    g2b = np.concatenate([np.zeros((64, 128), f32), G2], axis=0)
    wv2 = np.concatenate([wv.T, wv.T], axis=0)     # [128, 64]
    b3 = np.concatenate([np.asarray(inputs["c_b3"], dtype=f32),
                         np.asarray(inputs["t_b3"], dtype=f32)])[:, None]

    shared = dict(
        cw1=np.ascontiguousarray(np.asarray(inputs["c_w1"], dtype=f32).T),
        cw2=np.ascontiguousarray(np.asarray(inputs["c_w2"], dtype=f32).T),
        tw1=np.ascontiguousarray(np.asarray(inputs["t_w1"], dtype=f32).T),
        tw2=np.ascontiguousarray(np.asarray(inputs["t_w2"], dtype=f32).T),
        w3p=w3p, cfwp=cfwp, g2t=g2t, g2b=g2b, wv2=wv2,
        b1c=np.asarray(inputs["c_b1"], dtype=f32).reshape(KC, 128),
        b2c=np.asarray(inputs["c_b2"], dtype=f32).reshape(2, 128),
        b1t=np.asarray(inputs["t_b1"], dtype=f32).reshape(KT_, 128),
        b2t=np.asarray(inputs["t_b2"], dtype=f32).reshape(2, 128),
        b3=b3,
        bcf=np.asarray(inputs["cf_b"], dtype=f32)[:, None],
        **_build_consts(),
    )

    in_maps = []
    for c in range(N_CORES):
        idx = seq[c * (B // N_CORES):(c + 1) * (B // N_CORES)].reshape(-1)
        if T >= TOK:
            idx = np.concatenate([idx, np.zeros(T - TOK, np.int64)])
        else:
            idx = idx[:T]
        m = dict(shared)
        m["xc"] = np.ascontiguousarray(cf_full[idx].T)
        m["xt"] = np.ascontiguousarray(tf_full[idx].T)
        m["xcf"] = np.ascontiguousarray(cff_full[idx].T)
        m["xid"] = np.ascontiguousarray(ide_full[idx].T)
        in_maps.append(m)
    return in_maps


def _get_runner():
    """Cached jitted shard_map runner over 8 cores (mirrors
    bass2jax.run_bass_via_pjrt but reuses one jit so repeat calls skip
    retracing). Returns (fn, in_names, out_shape) where fn(concat_inputs_list)
    -> concatenated out array [8*64, T]."""
    if "runner" in _CACHE:
        return _CACHE["runner"]
    import jax
    from jax.sharding import Mesh, PartitionSpec
    try:
        from jax.experimental.shard_map import shard_map
    except ImportError:
        from jax.shard_map import shard_map
    from concourse import bass2jax, mybir

    nc = _get_nc()
    bass2jax.install_neuronx_cc_hook()
    partition_name = (nc.partition_id_tensor.name
                      if nc.partition_id_tensor else None)
    in_names, out_names, out_avals, zero_shapes = [], [], [], []
    for alloc in nc.m.functions[0].allocations:
        if not isinstance(alloc, mybir.MemoryLocationSet):
            continue
        name = alloc.memorylocations[0].name
        if alloc.kind == "ExternalInput":
            if name != partition_name:
                in_names.append(name)
        elif alloc.kind == "ExternalOutput":
            out_names.append(name)
            shape = tuple(alloc.tensor_shape)
            dtype = mybir.dt.np(alloc.dtype)
            out_avals.append(jax.core.ShapedArray(shape, dtype))
            zero_shapes.append((shape, dtype))
    n_params = len(in_names)
    full_in_names = list(in_names) + list(out_names)
    if partition_name is not None:
        full_in_names.append(partition_name)

    def _body(*args):
        operands = list(args)
        if partition_name is not None:
            operands.append(bass2jax.partition_id_tensor())
        outs = bass2jax._bass_exec_p.bind(
            *operands,
            out_avals=tuple(out_avals),
            in_names=tuple(full_in_names),
            out_names=tuple(out_names),
            lowering_input_output_aliases=(),
            sim_require_finite=True,
            sim_require_nnan=True,
            nc=nc,
        )
        return tuple(outs)

    devices = jax.devices()[:N_CORES]
    mesh = Mesh(np.asarray(devices), ("core",))
    n_outs = len(out_names)
    in_specs = (PartitionSpec("core"),) * (n_params + n_outs)
    out_specs = (PartitionSpec("core"),) * n_outs
    sharded = jax.jit(
        shard_map(_body, mesh=mesh, in_specs=in_specs, out_specs=out_specs,
                  check_rep=False),
        keep_unused=True,
    )
    runner = (sharded, in_names, out_names, zero_shapes, mesh)
    _CACHE["runner"] = runner
    return runner


def _run_device(in_maps):
    sharded, in_names, out_names, zero_shapes, _ = _get_runner()
    concat_in = [
        np.concatenate([np.asarray(in_maps[c][n]) for c in range(N_CORES)],
                       axis=0)
        for n in in_names
    ]
    concat_zeros = [np.zeros((N_CORES * s[0], *s[1:]), d)
                    for (s, d) in zero_shapes]
    out_arrs = sharded(*concat_in, *concat_zeros)
    return np.asarray(out_arrs[out_names.index("out")])


def kernel(**inputs):
    in_maps = _host_prep(inputs)
    out_cat = _run_device(in_maps)          # [8*64, T]
    rows = B // N_CORES
    full = np.empty((B, S, D), np.float32)
    for c in range(N_CORES):
        o = out_cat[c * 64:(c + 1) * 64, :TOK]   # [64, 6400]
        full[c * rows:(c + 1) * rows] = o.T.reshape(rows, S, D)
    return full


# revision 42
# speedup vs baseline: 57.0699x; 57.0699x over previous
"""Trainium2 Bass kernel for nn_DisentangledHierarchicalEncoder.

Strategy (8 NeuronCores, SPMD, zero collectives):
  The gather indices (seq_modify) are host-known, so the host pre-gathers the
  per-token raw features for each core's 6400 tokens (128 batch rows x 50) and
  pre-transposes everything to feature-major [feat, token] layout. Each core
  then runs a fully dense pipeline:
      content MLP (1024->1024->256->64) with input l2norm,
      text MLP (768->768->256->64) with input l2norm,
      cf linear (64->64), id passthrough,
      per-(token, modality) l2norm + LayerNorm (folded into one affine),
      4x4 self-attention (scores via G = 0.125 * wq.T @ wk), mean-pool,
  in 13 chunks of 512 tokens.  All matmuls run in float32r (TF32-like, 1
  cycle/row).  Per-token scalars are broadcast across partitions with PE
  outer-products against host-supplied pick matrices; partition-stacked pairs
  (content|text and cf|id) keep every engine op on 128 partitions.
"""

import numpy as np

NUM_ITEM = 50000
B, S, D = 1024, 50, 64
DC, DT = 1024, 768
N_CORES = 8
TOK = (B // N_CORES) * S          # 6400 real tokens per core
C = 512                           # chunk width (tokens per chunk)
NCH = 13                          # chunks per core
T = C * NCH                       # 6656 padded tokens per core
KC, KT_ = DC // 128, DT // 128    # k-tiles: 8 content, 6 text
LN_EPS = 1e-5
MLP_BF16 = False                  # L1/L2 matmuls in bf16 (FWL), rest float32r

_CACHE = {}


def _bf():
    if not MLP_BF16:
        return np.float32
    import ml_dtypes
    return ml_dtypes.bfloat16


def _build_consts():
    f32 = np.float32
    ones128 = np.ones((128, 1), f32)
    ones1 = np.ones((1, 128), f32)
    # stats lhsT [128, 8]. First matmul uses cols 0:4 on emb_ct (rows 0,1 of
    # st), second uses cols 4:8 on emb_cfid; within that slice cols 2,3 must
    # be the hot ones so the sums land on st rows 2,3.
    hotab = np.zeros((128, 8), f32)
    hotab[0:64, 0] = 1.0      # st row 0 <- sum over partitions 0:64 (c)
    hotab[64:128, 1] = 1.0    # st row 1 (t)
    hotab[0:64, 4 + 2] = 1.0  # st row 2 (cf)
    hotab[64:128, 4 + 3] = 1.0  # st row 3 (id)
    # score lhsT: 4 variants [128, 8]; variant u has slice-local cols
    # (2u, 2u+1) hot for (top, bottom) halves.
    hotp = np.zeros((128, 32), f32)
    for u in range(4):
        hotp[0:64, 8 * u + 2 * u] = 1.0
        hotp[64:128, 8 * u + 2 * u + 1] = 1.0
    # pick: [4, 256]; cols 0:128 broadcast rows 0/1 to halves, cols 128:256 rows 2/3
    pickp = np.zeros((4, 256), f32)
    pickp[0, 0:64] = 1.0
    pickp[1, 64:128] = 1.0
    pickp[2, 128 + 0:128 + 64] = 1.0
    pickp[3, 128 + 64:128 + 128] = 1.0
    # sumexp lhsT [8, 8]: first mm cols 0:4 on e_ct (se rows 0,1), second mm
    # cols 4:8 on e_cfid (slice-local cols 2,3 hot -> se rows 2,3).
    quads = np.zeros((8, 8), f32)
    quads[0:4, 0] = 1.0
    quads[4:8, 1] = 1.0
    quads[0:4, 4 + 2] = 1.0
    quads[4:8, 4 + 3] = 1.0
    # r replicate lhsT: [4, 16]; cols j (j<8): one-hot row j//4 (for e_ct pair);
    # cols 8+j: one-hot row 2 + j//4 (for e_cfid pair)
    reps = np.zeros((4, 16), f32)
    for j in range(8):
        reps[j // 4, j] = 1.0
        reps[2 + j // 4, 8 + j] = 1.0
    # a4 lhsT: [8, 4]; col n hot at rows {n, 4+n}
    nsums = np.zeros((8, 4), f32)
    for n in range(4):
        nsums[n, n] = 1.0
        nsums[4 + n, n] = 1.0
    return dict(ones128=ones128, ones1=ones1, hotab=hotab, hotp=hotp,
                pickp=pickp, quads=quads, reps=reps, nsums=nsums)


def _build_nc(nch=NCH, n_cores=N_CORES, repeat=1):
    import concourse.bacc as bacc
    import concourse.tile as tile
    from concourse import mybir
    from contextlib import ExitStack

    T = C * nch
    FR = mybir.dt.float32r
    F32 = mybir.dt.float32
    BF = mybir.dt.bfloat16 if MLP_BF16 else FR
    AF = mybir.ActivationFunctionType

    nc = bacc.Bacc("TRN2", target_bir_lowering=False, debug=False,
                   num_devices=n_cores)

    din = {}
    def dt_in(name, shape, dt=FR):
        din[name] = nc.dram_tensor(name, list(shape), dt, kind="ExternalInput")
        return din[name]

    xc = dt_in("xc", [DC, T], BF)
    xt = dt_in("xt", [DT, T], BF)
    xcf = dt_in("xcf", [64, T])
    xid = dt_in("xid", [64, T])
    cw1 = dt_in("cw1", [DC, DC], BF)
    cw2 = dt_in("cw2", [DC, 256], BF)
    tw1 = dt_in("tw1", [DT, DT], BF)
    tw2 = dt_in("tw2", [DT, 256], BF)
    w3p = dt_in("w3p", [128, 4, 128])
    cfwp = dt_in("cfwp", [64, 128])
    g2t = dt_in("g2t", [128, 128])
    g2b = dt_in("g2b", [128, 128])
    wv2 = dt_in("wv2", [128, 64])
    b1c = dt_in("b1c", [KC, 128], F32)
    b2c = dt_in("b2c", [2, 128], F32)
    b1t = dt_in("b1t", [KT_, 128], F32)
    b2t = dt_in("b2t", [2, 128], F32)
    b3 = dt_in("b3", [128, 1], F32)
    bcf = dt_in("bcf", [64, 1], F32)
    ones128 = dt_in("ones128", [128, 1])
    ones1 = dt_in("ones1", [1, 128])
    hotab = dt_in("hotab", [128, 8])
    hotp = dt_in("hotp", [128, 32])
    pickp = dt_in("pickp", [4, 256])
    quads = dt_in("quads", [8, 8])
    reps = dt_in("reps", [4, 16])
    nsums = dt_in("nsums", [8, 4])
    out = nc.dram_tensor("out", [64, T], F32, kind="ExternalOutput")

    xc_r = xc.rearrange("(kt p) t -> p kt t", p=128)
    xt_r = xt.rearrange("(kt p) t -> p kt t", p=128)

    with nc.allow_low_precision("float32r tiles feed float32r matmuls by design"), \
            tile.TileContext(nc) as tc:
        with ExitStack() as ctx:
            wp = ctx.enter_context(tc.tile_pool(name="wp", bufs=1))
            xin = ctx.enter_context(tc.tile_pool(name="xin", bufs=1))
            h1p = ctx.enter_context(tc.tile_pool(name="h1p", bufs=1))
            h2p = ctx.enter_context(tc.tile_pool(name="h2p", bufs=1))
            sqp = ctx.enter_context(tc.tile_pool(name="sqp", bufs=2))
            tmpp = ctx.enter_context(tc.tile_pool(name="tmpp", bufs=2))
            embp = ctx.enter_context(tc.tile_pool(name="embp", bufs=2))
            xnp = ctx.enter_context(tc.tile_pool(name="xnp", bufs=2))
            tinyp = ctx.enter_context(tc.tile_pool(name="tinyp", bufs=4))
            outp = ctx.enter_context(tc.tile_pool(name="outp", bufs=2))
            pbig = ctx.enter_context(tc.tile_pool(name="pbig", bufs=4,
                                                  space="PSUM"))
            psmall = ctx.enter_context(tc.tile_pool(name="psmall", bufs=4,
                                                    space="PSUM"))

            # ---- resident weights / consts ----
            cw1s = wp.tile([128, KC, DC], BF)
            nc.sync.dma_start(out=cw1s, in_=cw1.rearrange("(kt p) m -> p kt m", p=128))
            cw2s = wp.tile([128, KC, 256], BF)
            nc.sync.dma_start(out=cw2s, in_=cw2.rearrange("(kt p) m -> p kt m", p=128))
            tw1s = wp.tile([128, KT_, DT], BF)
            nc.sync.dma_start(out=tw1s, in_=tw1.rearrange("(kt p) m -> p kt m", p=128))
            tw2s = wp.tile([128, KT_, 256], BF)
            nc.sync.dma_start(out=tw2s, in_=tw2.rearrange("(kt p) m -> p kt m", p=128))
            w3ps = wp.tile([128, 4, 128], FR)
            nc.sync.dma_start(out=w3ps, in_=w3p[:, :, :])
            cfwps = wp.tile([64, 128], FR)
            nc.sync.dma_start(out=cfwps, in_=cfwp[:, :])
            g2ts = wp.tile([128, 128], FR)
            nc.sync.dma_start(out=g2ts, in_=g2t[:, :])
            g2bs = wp.tile([128, 128], FR)
            nc.sync.dma_start(out=g2bs, in_=g2b[:, :])
            wv2s = wp.tile([128, 64], FR)
            nc.sync.dma_start(out=wv2s, in_=wv2[:, :])
            b1cs = wp.tile([128, KC], F32)
            nc.sync.dma_start(out=b1cs, in_=b1c.rearrange("m p -> p m"))
            b2cs = wp.tile([128, 2], F32)
            nc.sync.dma_start(out=b2cs, in_=b2c.rearrange("m p -> p m"))
            b1ts = wp.tile([128, KT_], F32)
            nc.sync.dma_start(out=b1ts, in_=b1t.rearrange("m p -> p m"))
            b2ts = wp.tile([128, 2], F32)
            nc.sync.dma_start(out=b2ts, in_=b2t.rearrange("m p -> p m"))
            b3s = wp.tile([128, 1], F32)
            nc.sync.dma_start(out=b3s, in_=b3[:, :])
            bcfs = wp.tile([64, 1], F32)
            nc.sync.dma_start(out=bcfs, in_=bcf[:, :])
            o128 = wp.tile([128, 1], FR)
            nc.sync.dma_start(out=o128, in_=ones128[:, :])
            o1 = wp.tile([1, 128], FR)
            nc.sync.dma_start(out=o1, in_=ones1[:, :])
            hotabs = wp.tile([128, 8], FR)
            nc.sync.dma_start(out=hotabs, in_=hotab[:, :])
            hotps = wp.tile([128, 32], FR)
            nc.sync.dma_start(out=hotps, in_=hotp[:, :])
            pickps = wp.tile([4, 256], FR)
            nc.sync.dma_start(out=pickps, in_=pickp[:, :])
            quadss = wp.tile([8, 8], FR)
            nc.sync.dma_start(out=quadss, in_=quads[:, :])
            repss = wp.tile([4, 16], FR)
            nc.sync.dma_start(out=repss, in_=reps[:, :])
            nsumss = wp.tile([8, 4], FR)
            nc.sync.dma_start(out=nsumss, in_=nsums[:, :])

            from concourse.alu_op_type import AluOpType as ALU

            state = {}

            def make_units(j):
                """PE-dense MLP work for chunk j, as a list of emit fns."""
                sl = slice(j * C, (j + 1) * C)
                ctx_j = {}

                def u_load():
                    xc_j = xin.tile([128, KC, C], BF, tag="xc")
                    nc.sync.dma_start(out=xc_j, in_=xc_r[:, :, sl])
                    xt_j = xin.tile([128, KT_, C], BF, tag="xt")
                    nc.sync.dma_start(out=xt_j, in_=xt_r[:, :, sl])
                    xcf_j = xin.tile([64, C], FR, tag="xcf")
                    nc.sync.dma_start(out=xcf_j, in_=xcf[:, sl])
                    emb_cfid = embp.tile([128, C], FR, tag="emb_cfid")
                    nc.sync.dma_start(out=emb_cfid[64:128, :], in_=xid[:, sl])
                    ctx_j.update(xc_j=xc_j, xt_j=xt_j, xcf_j=xcf_j,
                                 emb_cfid=emb_cfid)

                def norm_ss(xj, kt, tag):
                    # squares on GPSIMD, ss via ones-matmul, inv = 1/max(sqrt)
                    ss_ps = psmall.tile([1, C], F32, tag="sm")
                    for k in range(kt):
                        sq_k = sqp.tile([128, C], FR, tag="sq", name="sq_k")
                        nc.gpsimd.tensor_mul(sq_k, xj[:, k, :], xj[:, k, :])
                        nc.tensor.matmul(ss_ps[:, :], o128[:, :], sq_k,
                                         start=(k == 0), stop=(k == kt - 1))
                    nrm = tinyp.tile([1, C], FR, tag="t1c", bufs=3)
                    nc.scalar.activation(out=nrm, in_=ss_ps[:, :], func=AF.Sqrt)
                    ncl = tinyp.tile([1, C], FR, tag="t1c", bufs=3)
                    nc.vector.tensor_scalar_max(ncl, nrm, 1e-12)
                    inv = tinyp.tile([1, C], FR, tag="t1c", bufs=3)
                    nc.vector.reciprocal(inv, ncl)
                    ctx_j["inv" + tag] = inv

                def norm_ib(tag):
                    # broadcast inv to [128, C] (emitted after L1 m0 so the
                    # sqrt/max/recip latency hides under matmul work)
                    ib_ps = pbig.tile([128, C], F32, tag="mm")
                    nc.tensor.matmul(ib_ps[:, :], o1[:, :],
                                     ctx_j["inv" + tag][:, :],
                                     start=True, stop=True)
                    invb = tmpp.tile([128, C], FR, tag="invb")
                    nc.vector.tensor_copy(invb, ib_ps[:, :])
                    ctx_j["invb" + tag] = invb

                def l1_tile(xj_key, kt, w1s, b1sT, tag, m):
                    def evac(ps, mi):
                        h1 = ctx_j["h1" + tag]
                        tmp = tmpp.tile([128, C], FR, tag="tmp", name="tmp")
                        nc.vector.tensor_mul(tmp, ps[:, :],
                                             ctx_j["invb" + tag])
                        nc.scalar.activation(out=h1[:, mi, :], in_=tmp,
                                             func=AF.Relu,
                                             bias=b1sT[:, mi:mi + 1])

                    def emit():
                        xj = ctx_j[xj_key]
                        if m == 0:
                            h1 = h1p.tile([128, kt, C], BF, tag="h1" + tag)
                            ctx_j["h1" + tag] = h1
                        ps = pbig.tile([128, C], F32, tag="mm", name="ps")
                        for k in range(kt):
                            nc.tensor.matmul(
                                ps[:, :], w1s[:, k, 128 * m:128 * (m + 1)],
                                xj[:, k, :], start=(k == 0), stop=(k == kt - 1))
                        if m == 0:
                            ctx_j["ps0" + tag] = ps
                            return
                        if m == 1:
                            norm_ib(tag)
                            evac(ctx_j.pop("ps0" + tag), 0)
                        evac(ps, m)
                    return emit

                def l2_tile(kt, w2s, b2sT, tag, m):
                    def emit():
                        h1 = ctx_j["h1" + tag]
                        if m == 0:
                            h2 = h2p.tile([128, 2, C], FR, tag="h2" + tag)
                            ctx_j["h2" + tag] = h2
                        h2 = ctx_j["h2" + tag]
                        ps = pbig.tile([128, C], F32, tag="mm")
                        for k in range(kt):
                            nc.tensor.matmul(
                                ps[:, :], w2s[:, k, 128 * m:128 * (m + 1)],
                                h1[:, k, :], start=(k == 0), stop=(k == kt - 1))
                        nc.scalar.activation(out=h2[:, m, :], in_=ps[:, :],
                                             func=AF.Relu,
                                             bias=b2sT[:, m:m + 1])
                    return emit

                def u_l3cf():
                    h2c, h2t = ctx_j["h2c"], ctx_j["h2t"]
                    ps3 = pbig.tile([128, C], F32, tag="mm")
                    nc.tensor.matmul(ps3[:, :], w3ps[:, 0, :], h2c[:, 0, :],
                                     start=True, stop=False)
                    nc.tensor.matmul(ps3[:, :], w3ps[:, 1, :], h2c[:, 1, :],
                                     start=False, stop=False)
                    nc.tensor.matmul(ps3[:, :], w3ps[:, 2, :], h2t[:, 0, :],
                                     start=False, stop=False)
                    nc.tensor.matmul(ps3[:, :], w3ps[:, 3, :], h2t[:, 1, :],
                                     start=False, stop=True)
                    emb_ct = embp.tile([128, C], FR, tag="emb_ct")
                    nc.scalar.activation(out=emb_ct, in_=ps3[:, :],
                                         func=AF.Identity, bias=b3s[:, :])
                    pcf = pbig.tile([128, C], F32, tag="mm")
                    nc.tensor.matmul(pcf[:, :], cfwps[:, :], ctx_j["xcf_j"],
                                     start=True, stop=True)
                    emb_cfid = ctx_j["emb_cfid"]
                    nc.scalar.activation(out=emb_cfid[0:64, :],
                                         in_=pcf[0:64, :],
                                         func=AF.Identity, bias=bcfs[:, :])
                    state[j] = (emb_ct, emb_cfid)

                units = [u_load,
                         lambda: norm_ss(ctx_j["xc_j"], KC, "c")]
                units += [l1_tile("xc_j", KC, cw1s, b1cs, "c", m)
                          for m in range(KC)]
                units += [lambda: norm_ss(ctx_j["xt_j"], KT_, "t")]
                units += [l2_tile(KC, cw2s, b2cs, "c", m) for m in range(2)]
                units += [l1_tile("xt_j", KT_, tw1s, b1ts, "t", m)
                          for m in range(KT_)]
                units += [l2_tile(KT_, tw2s, b2ts, "t", m) for m in range(2)]
                units += [u_l3cf]
                return units

            def make_stages(j):
                """attention for chunk j (embs from state[j]), as emit fns."""
                sl = slice(j * C, (j + 1) * C)
                actx = {}

                def s_stats():
                    emb_ct, emb_cfid = state.pop(j)
                    actx["emb_ct"], actx["emb_cfid"] = emb_ct, emb_cfid
                    sq_ct = sqp.tile([128, C], FR, tag="sqs")
                    nc.gpsimd.tensor_mul(sq_ct, emb_ct, emb_ct)
                    sq_cfid = sqp.tile([128, C], FR, tag="sqs")
                    nc.gpsimd.tensor_mul(sq_cfid, emb_cfid, emb_cfid)
                    st_sum = psmall.tile([4, C], F32, tag="sm")
                    nc.tensor.matmul(st_sum[:, :], hotabs[:, 0:4], emb_ct,
                                     start=True, stop=False)
                    nc.tensor.matmul(st_sum[:, :], hotabs[:, 4:8], emb_cfid,
                                     start=False, stop=True)
                    st_ss = psmall.tile([4, C], F32, tag="sm")
                    nc.tensor.matmul(st_ss[:, :], hotabs[:, 0:4], sq_ct,
                                     start=True, stop=False)
                    nc.tensor.matmul(st_ss[:, :], hotabs[:, 4:8], sq_cfid,
                                     start=False, stop=True)
                    actx["st_sum"], actx["st_ss"] = st_sum, st_ss

                def s_ab():
                    # folded l2norm+LN: A = rsqrt(ss*(1/64+eps) - mu^2), B = mu*A
                    mu4 = tinyp.tile([4, C], FR, tag="t4c")
                    nc.vector.tensor_scalar_mul(mu4, actx["st_sum"][:, :],
                                                1.0 / 64)
                    musq = tinyp.tile([4, C], FR, tag="t4c")
                    nc.gpsimd.tensor_mul(musq, mu4, mu4)
                    apre = tinyp.tile([4, C], FR, tag="t4c")
                    nc.vector.scalar_tensor_tensor(
                        apre, actx["st_ss"][:, :], 1.0 / 64 + LN_EPS, musq,
                        op0=ALU.mult, op1=ALU.subtract)
                    asq = tinyp.tile([4, C], FR, tag="t4c")
                    nc.scalar.activation(out=asq, in_=apre, func=AF.Sqrt)
                    A4 = tinyp.tile([4, C], FR, tag="t4c")
                    nc.vector.reciprocal(A4, asq)
                    B4 = tinyp.tile([4, C], FR, tag="t4c")
                    nc.vector.tensor_mul(B4, mu4, A4)
                    actx["A4"], actx["B4"] = A4, B4

                def make_xn(ekey, pks, tag):
                    def emit():
                        abp = pbig.tile([128, C], F32, tag="mm")
                        nc.tensor.matmul(abp[:, :], pickps[:, pks],
                                         actx["A4"], start=True, stop=True)
                        bbp = pbig.tile([128, C], F32, tag="mm")
                        nc.tensor.matmul(bbp[:, :], pickps[:, pks],
                                         actx["B4"], start=True, stop=True)
                        tmp2 = tmpp.tile([128, C], FR, tag="tmp")
                        nc.vector.tensor_mul(tmp2, actx[ekey], abp[:, :])
                        xn = xnp.tile([128, C], FR, tag="xn" + tag)
                        nc.vector.tensor_sub(xn, tmp2, bbp[:, :])
                        actx["xn" + tag] = xn
                    return emit

                def score_a(i):
                    def emit():
                        if i == 0:
                            actx["s_ct"] = psmall.tile([8, C], F32, tag="sm",
                                                       name="s_ct")
                            actx["s_cfid"] = psmall.tile([8, C], F32, tag="sm",
                                                         name="s_cfid")
                        gsel = g2ts if i % 2 == 0 else g2bs
                        xn_src = actx["xnct"] if i < 2 else actx["xncfid"]
                        qg = pbig.tile([128, C], F32, tag="mm", name="qg")
                        nc.tensor.matmul(qg[:, :], gsel[:, :], xn_src,
                                         start=True, stop=True)
                        pr1 = tmpp.tile([128, C], FR, tag="pr", name="pr1")
                        nc.vector.tensor_mul(pr1, qg[:, :], actx["xnct"])
                        pr2 = tmpp.tile([128, C], FR, tag="pr", name="pr2")
                        nc.vector.tensor_mul(pr2, qg[:, :], actx["xncfid"])
                        actx["pr"] = (pr1, pr2)
                    return emit

                def score_b(i):
                    def emit():
                        pr1, pr2 = actx.pop("pr")
                        s_tile = actx["s_ct"] if i < 2 else actx["s_cfid"]
                        u = 2 * (i % 2)
                        nc.tensor.matmul(s_tile[:, :],
                                         hotps[:, 8 * u:8 * u + 8],
                                         pr1, start=(i % 2 == 0), stop=False)
                        nc.tensor.matmul(s_tile[:, :],
                                         hotps[:, 8 * (u + 1):8 * (u + 1) + 8],
                                         pr2, start=False, stop=(i % 2 == 1))
                    return emit

                def s_soft1():
                    e_ct = tmpp.tile([8, C], FR, tag="e_ct", bufs=1)
                    nc.scalar.activation(out=e_ct, in_=actx["s_ct"][:, :],
                                         func=AF.Exp)
                    e_cfid = tmpp.tile([8, C], FR, tag="e_cfid", bufs=1)
                    nc.scalar.activation(out=e_cfid, in_=actx["s_cfid"][:, :],
                                         func=AF.Exp)
                    se = psmall.tile([4, C], F32, tag="sm")
                    nc.tensor.matmul(se[:, :], quadss[:, 0:4], e_ct,
                                     start=True, stop=False)
                    nc.tensor.matmul(se[:, :], quadss[:, 4:8], e_cfid,
                                     start=False, stop=True)
                    tse = tinyp.tile([4, C], FR, tag="t4c")
                    nc.vector.tensor_scalar_mul(tse, se[:, :], 4.0)
                    rr = tinyp.tile([4, C], FR, tag="t4c")
                    nc.vector.reciprocal(rr, tse)
                    actx.update(e_ct=e_ct, e_cfid=e_cfid, rr=rr)

                def s_soft2():
                    w_pair = []
                    for pi, e_p in enumerate([actx["e_ct"], actx["e_cfid"]]):
                        rrep = pbig.tile([128, C], F32, tag="mm", name="rrep")
                        nc.tensor.matmul(rrep[0:8, :],
                                         repss[:, 8 * pi:8 * pi + 8],
                                         actx["rr"], start=True, stop=True)
                        w_p = tmpp.tile([8, C], FR, tag="w_p", name="w_p",
                                        bufs=2)
                        nc.vector.tensor_mul(w_p, e_p, rrep[0:8, :])
                        w_pair.append(w_p)
                    actx["w_pair"] = w_pair

                def s_out1():
                    w_pair = actx["w_pair"]
                    a4p = psmall.tile([4, C], F32, tag="sm")
                    nc.tensor.matmul(a4p[:, :], nsumss[:, :], w_pair[0],
                                     start=True, stop=False)
                    nc.tensor.matmul(a4p[:, :], nsumss[:, :], w_pair[1],
                                     start=False, stop=True)
                    a4s = tinyp.tile([4, C], FR, tag="t4c")
                    nc.vector.tensor_copy(a4s, a4p[:, :])
                    actx["a4s"] = a4s

                def s_out2():
                    a4s = actx["a4s"]
                    zps = []
                    for pi, xn_p in enumerate([actx["xnct"], actx["xncfid"]]):
                        arp = pbig.tile([128, C], F32, tag="mm", name="arp")
                        nc.tensor.matmul(
                            arp[:, :],
                            pickps[:, slice(128 * pi, 128 * (pi + 1))],
                            a4s, start=True, stop=True)
                        zp = tmpp.tile([128, C], FR, tag="pr", name="zp")
                        nc.vector.tensor_mul(zp, xn_p, arp[:, :])
                        zps.append(zp)
                    actx["zps"] = zps

                def s_out3():
                    fps = pbig.tile([128, C], F32, tag="mm")
                    for pi, zp in enumerate(actx["zps"]):
                        nc.tensor.matmul(fps[0:64, :], wv2s[:, :], zp,
                                         start=(pi == 0), stop=(pi == 1))
                    out_sb = outp.tile([64, C], F32, tag="osb")
                    nc.vector.tensor_copy(out_sb, fps[0:64, :])
                    nc.sync.dma_start(out=out[:, sl], in_=out_sb)

                return [s_stats, s_ab,
                        make_xn("emb_ct", slice(0, 128), "ct"),
                        make_xn("emb_cfid", slice(128, 256), "cfid"),
                        score_a(0), score_b(0), score_a(1), score_b(1),
                        score_a(2), score_b(2), score_a(3), score_b(3),
                        s_soft1, s_soft2, s_out1, s_out2, s_out3]

            import os
            skip_attn = bool(int(os.environ.get("K_SKIP_ATTN", "0")))
            skip_mlp = bool(int(os.environ.get("K_SKIP_MLP", "0")))

            def emit_all():
                for it in range(nch + 1):
                    units = make_units(it) if it < nch else []
                    stages = (make_stages(it - 1) if it >= 1 else [])
                    if skip_attn:
                        stages = []
                        if it >= 1:
                            state.pop(it - 1, None)
                    if skip_mlp:
                        units = units[:1]  # loads only
                        stages = []
                    # weave: distribute stages evenly among units
                    n_u, n_s = len(units), len(stages)
                    si = 0
                    for ui, u in enumerate(units):
                        u()
                        want = ((ui + 1) * n_s) // max(n_u, 1)
                        while si < want:
                            stages[si]()
                            si += 1
                    while si < n_s:
                        stages[si]()
                        si += 1

            if repeat == 1:
                emit_all()
            else:
                with tc.For_i(0, repeat, 1):
                    emit_all()

    nc.finalize()
    return nc


def _get_nc():
    if "nc" not in _CACHE:
        _CACHE["nc"] = _build_nc()
    return _CACHE["nc"]


def _host_prep(inputs, T=T):
    f32 = np.float32
    seq = np.asarray(inputs["seq_modify"])
    seq = np.where(seq == NUM_ITEM, 0, seq).astype(np.int64)  # [1024, 50]
    cf_full = np.asarray(inputs["content_feature"], dtype=f32)
    tf_full = np.asarray(inputs["text_feature"], dtype=f32)
    cff_full = np.asarray(inputs["cf_feature"], dtype=f32)
    ide_full = np.asarray(inputs["item_embeddings"], dtype=f32)

    c_w3 = np.asarray(inputs["c_w3"], dtype=f32)   # [64, 256]
    t_w3 = np.asarray(inputs["t_w3"], dtype=f32)
    cw3T = np.ascontiguousarray(c_w3.T)            # [256, 64]
    tw3T = np.ascontiguousarray(t_w3.T)
    w3p = np.zeros((128, 4, 128), f32)
    w3p[:, 0, 0:64] = cw3T[0:128]
    w3p[:, 1, 0:64] = cw3T[128:256]
    w3p[:, 2, 64:128] = tw3T[0:128]
    w3p[:, 3, 64:128] = tw3T[128:256]
    cf_w = np.asarray(inputs["cf_w"], dtype=f32)
    cfwp = np.zeros((64, 128), f32)
    cfwp[:, 0:64] = cf_w.T
    wq = np.asarray(inputs["wq"], dtype=f32)
    wk = np.asarray(inputs["wk"], dtype=f32)
    wv = np.asarray(inputs["wv"], dtype=f32)
    G = (wq.T @ wk) * (D ** -0.5)
    G2 = np.concatenate([G, G], axis=1)            # [64, 128]
    g2t = np.concatenate([G2, np.zeros((64, 128), f32)], axis=0)
    g2b = np.concatenate([np.zeros((64, 128), f32), G2], axis=0)
    wv2 = np.concatenate([wv.T, wv.T], axis=0)     # [128, 64]
    b3 = np.concatenate([np.asarray(inputs["c_b3"], dtype=f32),
                         np.asarray(inputs["t_b3"], dtype=f32)])[:, None]

    shared = dict(
        cw1=np.ascontiguousarray(np.asarray(inputs["c_w1"], dtype=f32).T
                                 .astype(_bf())),
        cw2=np.ascontiguousarray(np.asarray(inputs["c_w2"], dtype=f32).T
                                 .astype(_bf())),
        tw1=np.ascontiguousarray(np.asarray(inputs["t_w1"], dtype=f32).T
                                 .astype(_bf())),
        tw2=np.ascontiguousarray(np.asarray(inputs["t_w2"], dtype=f32).T
                                 .astype(_bf())),
        w3p=w3p, cfwp=cfwp, g2t=g2t, g2b=g2b, wv2=wv2,
        b1c=np.asarray(inputs["c_b1"], dtype=f32).reshape(KC, 128),
        b2c=np.asarray(inputs["c_b2"], dtype=f32).reshape(2, 128),
        b1t=np.asarray(inputs["t_b1"], dtype=f32).reshape(KT_, 128),
        b2t=np.asarray(inputs["t_b2"], dtype=f32).reshape(2, 128),
        b3=b3,
        bcf=np.asarray(inputs["cf_b"], dtype=f32)[:, None],
        **_build_consts(),
    )

    in_maps = []
    for c in range(N_CORES):
        idx = seq[c * (B // N_CORES):(c + 1) * (B // N_CORES)].reshape(-1)
        if T >= TOK:
            idx = np.concatenate([idx, np.zeros(T - TOK, np.int64)])
        else:
            idx = idx[:T]
        m = dict(shared)
        m["xc"] = np.ascontiguousarray(cf_full[idx].T.astype(_bf()))
        m["xt"] = np.ascontiguousarray(tf_full[idx].T.astype(_bf()))
        m["xcf"] = np.ascontiguousarray(cff_full[idx].T)
        m["xid"] = np.ascontiguousarray(ide_full[idx].T)
        in_maps.append(m)
    return in_maps


def _get_runner(nc=None, key="runner"):
    """Cached jitted shard_map runner over 8 cores (mirrors
    bass2jax.run_bass_via_pjrt but reuses one jit so repeat calls skip
    retracing)."""
    if key in _CACHE:
        return _CACHE[key]
    import jax
    from jax.sharding import Mesh, PartitionSpec
    try:
        from jax.experimental.shard_map import shard_map
    except ImportError:
        from jax.shard_map import shard_map
    from concourse import bass2jax, mybir

    if nc is None:
        nc = _get_nc()
    bass2jax.install_neuronx_cc_hook()
    partition_name = (nc.partition_id_tensor.name
                      if nc.partition_id_tensor else None)
    in_names, out_names, out_avals, zero_shapes = [], [], [], []
    for alloc in nc.m.functions[0].allocations:
        if not isinstance(alloc, mybir.MemoryLocationSet):
            continue
        name = alloc.memorylocations[0].name
        if alloc.kind == "ExternalInput":
            if name != partition_name:
                in_names.append(name)
        elif alloc.kind == "ExternalOutput":
            out_names.append(name)
            shape = tuple(alloc.tensor_shape)
            dtype = mybir.dt.np(alloc.dtype)
            out_avals.append(jax.core.ShapedArray(shape, dtype))
            zero_shapes.append((shape, dtype))
    n_params = len(in_names)
    full_in_names = list(in_names) + list(out_names)
    if partition_name is not None:
        full_in_names.append(partition_name)

    def _body(*args):
        operands = list(args)
        if partition_name is not None:
            operands.append(bass2jax.partition_id_tensor())
        outs = bass2jax._bass_exec_p.bind(
            *operands,
            out_avals=tuple(out_avals),
            in_names=tuple(full_in_names),
            out_names=tuple(out_names),
            lowering_input_output_aliases=(),
            sim_require_finite=True,
            sim_require_nnan=True,
            nc=nc,
        )
        return tuple(outs)

    devices = jax.devices()[:N_CORES]
    mesh = Mesh(np.asarray(devices), ("core",))
    n_outs = len(out_names)
    in_specs = (PartitionSpec("core"),) * (n_params + n_outs)
    out_specs = (PartitionSpec("core"),) * n_outs
    sharded = jax.jit(
        shard_map(_body, mesh=mesh, in_specs=in_specs, out_specs=out_specs,
                  check_rep=False),
        keep_unused=True,
    )
    runner = (sharded, in_names, out_names, zero_shapes, mesh)
    _CACHE[key] = runner
    return runner


def _run_device(in_maps):
    sharded, in_names, out_names, zero_shapes, _ = _get_runner()
    concat_in = [
        np.concatenate([np.asarray(in_maps[c][n]) for c in range(N_CORES)],
                       axis=0)
        for n in in_names
    ]
    concat_zeros = [np.zeros((N_CORES * s[0], *s[1:]), d)
                    for (s, d) in zero_shapes]
    out_arrs = sharded(*concat_in, *concat_zeros)
    return np.asarray(out_arrs[out_names.index("out")])


def kernel(**inputs):
    in_maps = _host_prep(inputs)
    out_cat = _run_device(in_maps)          # [8*64, T]
    rows = B // N_CORES
    full = np.empty((B, S, D), np.float32)
    for c in range(N_CORES):
        o = out_cat[c * 64:(c + 1) * 64, :TOK]   # [64, 6400]
        full[c * rows:(c + 1) * rows] = o.T.reshape(rows, S, D)
    return full
